# revision 3
# baseline (speedup 1.0000x reference)
"""LM head log_softmax kernel for 8 Trainium2 NeuronCores.

Computes log_softmax(h @ W^T) for h [2,2048,1024] f32, W [50257,1024] f32.

Strategy (tensor parallel over vocab), v2:
  - W sharded along vocab across 8 cores (6400 padded cols each, 51200 total).
  - Per m-tile (128 tokens), logits are computed in fp8 DoubleRow matmuls into
    wide [128, 2048] PSUM groups (4 banks), then drained by TWO independent
    readers:
      * scalar engine: Exp with per-group accumulation (row sums); the exp
        values themselves go to a small rotating scratch and are discarded.
      * vector engine: tensor_scalar mult (1/W_SCALE) that stages the LOGITS
        as bf16 in SBUF.
  - Per 4-m-tile block, the per-row sums are all-reduced across the 8 cores
    ([128,4] f32 payload, 8 collectives total).
  - Pass 2 (lagged one block to hide collective latency): lse = Ln(gsums) on
    the scalar engine (tiny), then per m-tile a single in-place 4x-mode DVE
    tensor_scalar subtract (stage -= lse) and a bf16 DMA of the result.
  - Output travels as bf16 (halves the HBM write vs f32); host casts to f32.
    Quantization error ~1e-3 vs the 2e-2 gate.
  - Vocab padding (zero W rows -> logit 0 -> exp 1) is corrected by a host
    supplied additive adjustment to the local row sums (-n_pad on the last
    core), exact since exp(0) == 1.

Host side: transposes h and the W shard to K-major (fp8), launches the SPMD
kernel via run_bass_kernel_spmd on cores 0-7, concatenates the per-core
[4096, 6400] bf16 outputs along vocab, slices off the padding, casts to f32.
"""

import os

import numpy as np
import ml_dtypes

import concourse.bass as bass
import concourse.bacc as bacc
import concourse.mybir as mybir
import concourse.tile as tile
from concourse.bass_utils import run_bass_kernel_spmd

N_CORES = 8
B, S, D = 2, 2048, 1024
T = B * S                      # 4096 tokens
V = 50257
VC = 6400                      # per-core padded vocab shard (8*6400 = 51200)
P = 128                        # SBUF partitions
K_TILES = D // P               # 8
K_PAIRS = K_TILES // 2         # 4 (fp8 DoubleRow handles 2 k-tiles at once)
M_TILES = T // P               # 32
BLK_MT = 4                     # m-tiles (128 tokens each) per collective block
N_BLOCKS = M_TILES // BLK_MT   # 8
# PSUM drain groups per m-tile: wide activations amortize the per-instruction
# overhead (PSUM access setup + accumulator read)
GROUPS = [(0, 2048), (2048, 2048), (4096, 2048), (6144, 256)]

BF16 = mybir.dt.bfloat16
F32 = mybir.dt.float32
FP8 = mybir.dt.float8e4
NP_FP8 = mybir.dt.np(mybir.dt.float8e4)
NP_BF16 = mybir.dt.np(mybir.dt.bfloat16)
W_SCALE = 32.0

# results of the last run_bass_kernel_spmd call (for test harness inspection)
LAST_RESULT = None


def build_nc():
    nc = bacc.Bacc(
        "TRN2",
        target_bir_lowering=False,
        debug=False,
        num_devices=N_CORES,
    )
    hT = nc.dram_tensor("hT", [D, T], FP8, kind="ExternalInput").ap()
    wT = nc.dram_tensor("wT", [D, VC], FP8, kind="ExternalInput").ap()
    adj = nc.dram_tensor("adj", [P, 1], F32, kind="ExternalInput").ap()
    out = nc.dram_tensor("out", [T, VC], BF16, kind="ExternalOutput").ap()

    # K-major views with the partition dim innermost of K: [128, K_TILES, *]
    hT_r = hT.rearrange("(k p) m -> p k m", p=P)
    wT_r = wT.rearrange("(k p) n -> p k n", p=P)

    with tile.TileContext(nc) as tc:
        with (
            tc.tile_pool(name="singles", bufs=1) as singles,
            tc.tile_pool(name="hts", bufs=3) as hts_pool,
            tc.tile_pool(name="psum", bufs=2, space="PSUM") as psum_pool,
            tc.tile_pool(name="stage", bufs=2 * BLK_MT) as stage_pool,
            tc.tile_pool(name="scratch", bufs=2) as scratch_pool,
            tc.tile_pool(name="stats", bufs=6) as stats_pool,
            tc.tile_pool(name="cc", bufs=4, space="DRAM") as cc_pool,
        ):
            wt_sb = singles.tile([P, K_TILES, VC], FP8)
            for k in range(K_TILES):
                nc.sync.dma_start(out=wt_sb[:, k, :], in_=wT_r[:, k, :])
            adj_sb = singles.tile([P, 1], F32)
            nc.sync.dma_start(out=adj_sb, in_=adj)

            def emit_pass2(stage_tiles_p, gsums_p, blk_p):
                lse = stats_pool.tile([P, BLK_MT], F32, name="lse")
                nc.scalar.activation(
                    out=lse,
                    in_=gsums_p,
                    func=mybir.ActivationFunctionType.Ln,
                )
                for mb, stage_m in enumerate(stage_tiles_p):
                    m = blk_p * BLK_MT + mb
                    nc.vector.tensor_scalar(
                        out=stage_m[:, :],
                        in0=stage_m[:, :],
                        scalar1=lse[:, mb : mb + 1],
                        scalar2=None,
                        op0=mybir.AluOpType.subtract,
                    )
                    nc.sync.dma_start(
                        out=out[m * P : (m + 1) * P, :], in_=stage_m[:, :]
                    )

            pending = None
            for blk in range(N_BLOCKS):
                stage_tiles = []
                lsums = stats_pool.tile([P, BLK_MT], F32, name="lsums")
                for mb in range(BLK_MT):
                    m = blk * BLK_MT + mb
                    ht = hts_pool.tile([P, K_TILES, P], FP8)
                    nc.sync.dma_start(
                        out=ht, in_=hT_r[:, :, m * P : (m + 1) * P]
                    )
                    stage_m = stage_pool.tile([P, VC], BF16, tag="stage")
                    sums_acc = stats_pool.tile([P, len(GROUPS)], F32)
                    for g, (goff, gw) in enumerate(GROUPS):
                        ps = psum_pool.tile([P, 2048], F32, tag="ps")
                        # kp-outer keeps consecutive matmuls in different
                        # PSUM banks (ILP) and shares the stationary operand
                        for kp in range(K_PAIRS):
                            for j in range(0, gw, 512):
                                cs = min(512, gw - j)
                                nc.tensor.matmul(
                                    out=ps[:, j : j + cs],
                                    lhsT=ht[:, 2 * kp : 2 * kp + 2, :],
                                    rhs=wt_sb[
                                        :,
                                        2 * kp : 2 * kp + 2,
                                        goff + j : goff + j + cs,
                                    ],
                                    start=(kp == 0),
                                    stop=(kp == K_PAIRS - 1),
                                    perf_mode=mybir.MatmulPerfMode.DoubleRow,
                                )
                        exp_scr = scratch_pool.tile([P, 2048], BF16)
                        nc.scalar.activation(
                            out=exp_scr[:, :gw],
                            in_=ps[:, :gw],
                            func=mybir.ActivationFunctionType.Exp,
                            scale=1.0 / W_SCALE,
                            accum_out=sums_acc[:, g : g + 1],
                        )
                        nc.vector.tensor_scalar(
                            out=stage_m[:, goff : goff + gw],
                            in0=ps[:, :gw],
                            scalar1=1.0 / W_SCALE,
                            scalar2=None,
                            op0=mybir.AluOpType.mult,
                        )
                    red = stats_pool.tile([P, 1], F32, name="red")
                    nc.vector.tensor_reduce(
                        out=red,
                        in_=sums_acc,
                        axis=mybir.AxisListType.X,
                        op=mybir.AluOpType.add,
                    )
                    nc.vector.tensor_add(
                        out=lsums[:, mb : mb + 1], in0=red, in1=adj_sb
                    )
                    stage_tiles.append(stage_m)

                cc_in = cc_pool.tile([P, BLK_MT], F32, tag="cc_in")
                cc_out = cc_pool.tile([P, BLK_MT], F32, tag="cc_out")
                nc.gpsimd.dma_start(out=cc_in[:, :], in_=lsums[:, :])
                nc.gpsimd.collective_compute(
                    "AllReduce",
                    mybir.AluOpType.add,
                    replica_groups=[list(range(N_CORES))],
                    ins=[cc_in[:, :].opt()],
                    outs=[cc_out[:, :].opt()],
                )
                gsums = stats_pool.tile([P, BLK_MT], F32, name="gsums")
                nc.gpsimd.dma_start(out=gsums[:, :], in_=cc_out[:, :])

                # pipeline the epilogue one block back: pass2 of block b-1 is
                # emitted after block b's compute + collective issue, so the
                # AllReduce latency hides behind a full block of matmul/exp
                # and the Ln never head-of-line blocks upcoming Exp work
                if pending is not None:
                    emit_pass2(*pending)
                pending = (stage_tiles, gsums, blk)
            emit_pass2(*pending)
    nc.compile()
    return nc


def _prep_inputs(hidden_states, W):
    """Host-side shard + transpose + cast. Returns per-core input maps."""
    hflat = np.asarray(hidden_states, dtype=np.float32).reshape(T, D)
    hT = np.ascontiguousarray(hflat.T).astype(NP_FP8)

    W = np.asarray(W, dtype=np.float32)
    in_maps = []
    for c in range(N_CORES):
        lo, hi = c * VC, (c + 1) * VC
        shard = W[lo : min(hi, V)]
        n_pad = VC - shard.shape[0]
        wT_c = np.zeros((D, VC), dtype=NP_FP8)
        wT_c[:, : shard.shape[0]] = (shard.T * W_SCALE).astype(NP_FP8)
        adj_c = np.full((P, 1), -float(n_pad), dtype=np.float32)
        in_maps.append({"hT": hT, "wT": wT_c, "adj": adj_c})
    return in_maps


def kernel(hidden_states, W):
    global LAST_RESULT
    in_maps = _prep_inputs(hidden_states, W)
    nc = build_nc()
    trace = os.environ.get("LMHEAD_TRACE", "0") == "1"
    res = run_bass_kernel_spmd(
        nc, in_maps, list(range(N_CORES)), trace=trace
    )
    LAST_RESULT = res
    parts = [
        np.asarray(res.results[c]["out"]).astype(np.float32)
        for c in range(N_CORES)
    ]
    full = np.concatenate(parts, axis=1)[:, :V]
    return np.ascontiguousarray(full.reshape(B, S, V).astype(np.float32))


# revision 4
# speedup vs baseline: 1.1449x; 1.1449x over previous
"""LM head log_softmax kernel for 8 Trainium2 NeuronCores.

Computes log_softmax(h @ W^T) for h [2,2048,1024] f32, W [50257,1024] f32.

Strategy (tensor parallel over vocab), v2:
  - W sharded along vocab across 8 cores (6400 padded cols each, 51200 total).
  - Per m-tile (128 tokens), logits are computed in fp8 DoubleRow matmuls into
    wide [128, 2048] PSUM groups (4 banks), then drained by TWO independent
    readers:
      * scalar engine: Exp with per-group accumulation (row sums); the exp
        values themselves go to a small rotating scratch and are discarded.
      * vector engine: tensor_scalar mult (1/W_SCALE) that stages the LOGITS
        as bf16 in SBUF.
  - Per 4-m-tile block, the per-row sums are all-reduced across the 8 cores
    ([128,4] f32 payload, 8 collectives total).
  - Pass 2 (lagged one block to hide collective latency): lse = Ln(gsums) on
    the scalar engine (tiny), then per m-tile a single in-place 4x-mode DVE
    tensor_scalar subtract (stage -= lse) and a bf16 DMA of the result.
  - Output travels as bf16 (halves the HBM write vs f32); host casts to f32.
    Quantization error ~1e-3 vs the 2e-2 gate.
  - Vocab padding (zero W rows -> logit 0 -> exp 1) is corrected by a host
    supplied additive adjustment to the local row sums (-n_pad on the last
    core), exact since exp(0) == 1.

Host side: transposes h and the W shard to K-major (fp8), launches the SPMD
kernel via run_bass_kernel_spmd on cores 0-7, concatenates the per-core
[4096, 6400] bf16 outputs along vocab, slices off the padding, casts to f32.
"""

import os

import numpy as np
import ml_dtypes

import concourse.bass as bass
import concourse.bacc as bacc
import concourse.mybir as mybir
import concourse.tile as tile
from concourse.bass_utils import run_bass_kernel_spmd

N_CORES = 8
B, S, D = 2, 2048, 1024
T = B * S                      # 4096 tokens
V = 50257
VC = 6400                      # per-core padded vocab shard (8*6400 = 51200)
P = 128                        # SBUF partitions
K_TILES = D // P               # 8
K_PAIRS = K_TILES // 2         # 4 (fp8 DoubleRow handles 2 k-tiles at once)
M_TILES = T // P               # 32
BLK_MT = 4                     # m-tiles (128 tokens each) per collective block
N_BLOCKS = M_TILES // BLK_MT   # 8
# PSUM drain groups per m-tile: wide activations amortize the per-instruction
# overhead (PSUM access setup + accumulator read)
GROUPS = [(0, 2048), (2048, 2048), (4096, 2048), (6144, 256)]

BF16 = mybir.dt.bfloat16
F32 = mybir.dt.float32
FP8 = mybir.dt.float8e4
NP_FP8 = mybir.dt.np(mybir.dt.float8e4)
NP_BF16 = mybir.dt.np(mybir.dt.bfloat16)
W_SCALE = 32.0

# results of the last run_bass_kernel_spmd call (for test harness inspection)
LAST_RESULT = None


def build_nc():
    nc = bacc.Bacc(
        "TRN2",
        target_bir_lowering=False,
        debug=False,
        num_devices=N_CORES,
    )
    hT = nc.dram_tensor("hT", [D, T], FP8, kind="ExternalInput").ap()
    wT = nc.dram_tensor("wT", [D, VC], FP8, kind="ExternalInput").ap()
    adj = nc.dram_tensor("adj", [P, 1], F32, kind="ExternalInput").ap()
    out = nc.dram_tensor("out", [T, VC], BF16, kind="ExternalOutput").ap()

    # K-major views with the partition dim innermost of K: [128, K_TILES, *]
    hT_r = hT.rearrange("(k p) m -> p k m", p=P)
    wT_r = wT.rearrange("(k p) n -> p k n", p=P)

    with tile.TileContext(nc) as tc:
        with (
            tc.tile_pool(name="singles", bufs=1) as singles,
            tc.tile_pool(name="hts", bufs=3) as hts_pool,
            tc.tile_pool(name="psum", bufs=2, space="PSUM") as psum_pool,
            tc.tile_pool(name="stage", bufs=2 * BLK_MT) as stage_pool,
            tc.tile_pool(name="scratch", bufs=2) as scratch_pool,
            tc.tile_pool(name="stats", bufs=6) as stats_pool,
            tc.tile_pool(name="cc", bufs=4, space="DRAM") as cc_pool,
        ):
            wt_sb = singles.tile([P, K_TILES, VC], FP8)
            for k in range(K_TILES):
                nc.sync.dma_start(out=wt_sb[:, k, :], in_=wT_r[:, k, :])
            adj_sb = singles.tile([P, 1], F32)
            nc.sync.dma_start(out=adj_sb, in_=adj)

            def emit_pass2(stage_tiles_p, gsums_p, blk_p):
                lse = stats_pool.tile([P, BLK_MT], F32, name="lse")
                nc.scalar.activation(
                    out=lse,
                    in_=gsums_p,
                    func=mybir.ActivationFunctionType.Ln,
                )
                for mb, stage_m in enumerate(stage_tiles_p):
                    m = blk_p * BLK_MT + mb
                    nc.vector.tensor_scalar(
                        out=stage_m[:, :],
                        in0=stage_m[:, :],
                        scalar1=lse[:, mb : mb + 1],
                        scalar2=None,
                        op0=mybir.AluOpType.subtract,
                    )
                    nc.sync.dma_start(
                        out=out[m * P : (m + 1) * P, :], in_=stage_m[:, :]
                    )

            pending = None
            for blk in range(N_BLOCKS):
                stage_tiles = []
                lsums = stats_pool.tile([P, BLK_MT], F32, name="lsums")
                for mb in range(BLK_MT):
                    m = blk * BLK_MT + mb
                    ht = hts_pool.tile([P, K_TILES, P], FP8)
                    nc.sync.dma_start(
                        out=ht, in_=hT_r[:, :, m * P : (m + 1) * P]
                    )
                    stage_m = stage_pool.tile([P, VC], BF16, tag="stage")
                    sums_acc = stats_pool.tile([P, len(GROUPS)], F32)
                    for g, (goff, gw) in enumerate(GROUPS):
                        ps = psum_pool.tile([P, 2048], F32, tag="ps")
                        # kp-outer keeps consecutive matmuls in different
                        # PSUM banks (ILP) and shares the stationary operand
                        for kp in range(K_PAIRS):
                            for j in range(0, gw, 512):
                                cs = min(512, gw - j)
                                nc.tensor.matmul(
                                    out=ps[:, j : j + cs],
                                    lhsT=ht[:, 2 * kp : 2 * kp + 2, :],
                                    rhs=wt_sb[
                                        :,
                                        2 * kp : 2 * kp + 2,
                                        goff + j : goff + j + cs,
                                    ],
                                    start=(kp == 0),
                                    stop=(kp == K_PAIRS - 1),
                                    perf_mode=mybir.MatmulPerfMode.DoubleRow,
                                )
                        # DVE is the ONLY psum reader: stage the logits as
                        # bf16 (folding the 1/W_SCALE descale), freeing the
                        # psum bank after ~2.3us so the PE never stalls on
                        # bank reuse. Exp+accum then reads the staged bf16
                        # copy from SBUF, off the psum critical path.
                        nc.vector.tensor_scalar(
                            out=stage_m[:, goff : goff + gw],
                            in0=ps[:, :gw],
                            scalar1=1.0 / W_SCALE,
                            scalar2=None,
                            op0=mybir.AluOpType.mult,
                        )
                        exp_scr = scratch_pool.tile([P, 2048], BF16)
                        nc.scalar.activation(
                            out=exp_scr[:, :gw],
                            in_=stage_m[:, goff : goff + gw],
                            func=mybir.ActivationFunctionType.Exp,
                            accum_out=sums_acc[:, g : g + 1],
                        )
                    red = stats_pool.tile([P, 1], F32, name="red")
                    nc.vector.tensor_reduce(
                        out=red,
                        in_=sums_acc,
                        axis=mybir.AxisListType.X,
                        op=mybir.AluOpType.add,
                    )
                    nc.vector.tensor_add(
                        out=lsums[:, mb : mb + 1], in0=red, in1=adj_sb
                    )
                    stage_tiles.append(stage_m)

                cc_in = cc_pool.tile([P, BLK_MT], F32, tag="cc_in")
                cc_out = cc_pool.tile([P, BLK_MT], F32, tag="cc_out")
                nc.gpsimd.dma_start(out=cc_in[:, :], in_=lsums[:, :])
                nc.gpsimd.collective_compute(
                    "AllReduce",
                    mybir.AluOpType.add,
                    replica_groups=[list(range(N_CORES))],
                    ins=[cc_in[:, :].opt()],
                    outs=[cc_out[:, :].opt()],
                )
                gsums = stats_pool.tile([P, BLK_MT], F32, name="gsums")
                nc.gpsimd.dma_start(out=gsums[:, :], in_=cc_out[:, :])

                # pipeline the epilogue one block back: pass2 of block b-1 is
                # emitted after block b's compute + collective issue, so the
                # AllReduce latency hides behind a full block of matmul/exp
                # and the Ln never head-of-line blocks upcoming Exp work
                if pending is not None:
                    emit_pass2(*pending)
                pending = (stage_tiles, gsums, blk)
            emit_pass2(*pending)
    nc.compile()
    return nc


def _prep_inputs(hidden_states, W):
    """Host-side shard + transpose + cast. Returns per-core input maps."""
    hflat = np.asarray(hidden_states, dtype=np.float32).reshape(T, D)
    hT = np.ascontiguousarray(hflat.T).astype(NP_FP8)

    W = np.asarray(W, dtype=np.float32)
    in_maps = []
    for c in range(N_CORES):
        lo, hi = c * VC, (c + 1) * VC
        shard = W[lo : min(hi, V)]
        n_pad = VC - shard.shape[0]
        wT_c = np.zeros((D, VC), dtype=NP_FP8)
        wT_c[:, : shard.shape[0]] = (shard.T * W_SCALE).astype(NP_FP8)
        adj_c = np.full((P, 1), -float(n_pad), dtype=np.float32)
        in_maps.append({"hT": hT, "wT": wT_c, "adj": adj_c})
    return in_maps


def kernel(hidden_states, W):
    global LAST_RESULT
    in_maps = _prep_inputs(hidden_states, W)
    nc = build_nc()
    trace = os.environ.get("LMHEAD_TRACE", "0") == "1"
    res = run_bass_kernel_spmd(
        nc, in_maps, list(range(N_CORES)), trace=trace
    )
    LAST_RESULT = res
    parts = [
        np.asarray(res.results[c]["out"]).astype(np.float32)
        for c in range(N_CORES)
    ]
    full = np.concatenate(parts, axis=1)[:, :V]
    return np.ascontiguousarray(full.reshape(B, S, V).astype(np.float32))
